# revision 19
# baseline (speedup 1.0000x reference)
"""Single-head causal attention kernel for TRN2 (8 NeuronCores, data-parallel).

Problem: x[256,256,384] f32, Wq/Wk/Wv[384,64] f32 ->
  out = softmax(mask((x@Wq)(x@Wk)^T/8)) @ (x@Wv)  [256,256,64] f32

Sharding: batch 256 -> 8 cores x 32 batches. Weights replicated.

Host-side marshaling (inside kernel(), per core):
  x slice  -> bf16, transposed to xT layout [cc, c, b, t]  (c = cc*128+c')
  Wq|Wk    -> packed bf16 [cc, c, 128] (cols 0:64 q, 64:128 k)
  Wv       -> bf16 [cc, c, 64]
  tri2     -> [128,256] bf16 = [tri|tri] causal keep-mask (diag blocks)
  out      <- bf16 [t, b, h], host transposes back to [b, t, h] fp32

The xT layout kills all on-device PE transposes (projections contract C, so
both operands want C on partitions) and halves x HBM traffic vs fp32. All
DMA descriptors are >=1KB contiguous runs; all 8 x-group loads are issued
up-front (no waits) so the SP sequencer never head-of-line blocks them.

Per-core dataflow (bf16 matmuls, fp32 PSUM), per batch-pair:
  qkT [h2=128, 2, 256] = wqk^T @ xT    3 MMs (512-col rhs), PSUM->SBUF (DVE)
  kT -> partition base 0 via SBUF->SBUF DMA (PE operands must share array
        rows; engines cannot shift partitions, DMA can) - one per group
  v   [t, 64] = x @ Wv                 12 MMs (64-col rhs), ACT copy to va
  scT [s, t]  diag-adjacent cols:      6 MMs: 0:128=(s0,t0) 128:256=(s1,t1)
        256:384=(s0,t1); causal mask = ONE GpSimd mul of cols 0:256 by tri2
  mexp = exp(scT/8) on ACT (bf16 out)
  oa  [t, 65] = mexp^T @ [v|1]         6 MMs; rowsum lands in col 64
  normalize: DVE pair-reciprocal + broadcast-mul -> osb bf16

3-stage software pipeline keeps PE dense (HAM @ 2.4GHz): at iteration p the
kernel emits proj(p), scores(p-3), attv(p-4); cross-engine results are
always >=1 full iteration old when PE consumes them.
"""

import numpy as np
import ml_dtypes

B, T, C, H = 256, 256, 384, 64
NCORES = 8
BPC = B // NCORES  # 32 batches per core
CCH = C // 128  # 3 contraction chunks
TCH = T // 128  # 2 t-chunks
NB = 4  # batches per x-load group
NG = BPC // NB  # 8 groups
NP = BPC // 2  # 16 pairs
SG = 8  # batches per out-store super-group
LAG_SC = 3  # scores stage lag (pairs)
LAG_AV = 6  # attv stage lag (pairs)
WARMUP_MM = 17  # garbage 256-col matmuls to push HAM to 2.4GHz while x loads

BF16 = ml_dtypes.bfloat16

_CACHE = {}


def _build():
    import concourse.mybir as mybir
    import concourse.tile as tile
    from concourse import bacc

    fp32 = mybir.dt.float32
    bf16 = mybir.dt.bfloat16
    Exp = mybir.ActivationFunctionType.Exp
    Copy = mybir.ActivationFunctionType.Copy

    nc = bacc.Bacc()
    xt_d = nc.declare_dram_parameter("xt", [CCH, 128, BPC, T], bf16, isOutput=False)
    wqk_d = nc.declare_dram_parameter("wqk", [CCH, 128, 128], bf16, isOutput=False)
    wv_d = nc.declare_dram_parameter("wv", [CCH, 128, H], bf16, isOutput=False)
    tri_d = nc.declare_dram_parameter("tri2", [128, 256], bf16, isOutput=False)
    out_d = nc.declare_dram_parameter("out", [T, BPC, H], bf16, isOutput=True)

    with tile.TileContext(nc) as tc:
        with (
            tc.tile_pool(name="singles", bufs=1) as singles,
            tc.tile_pool(name="xin", bufs=8) as xin,
            tc.tile_pool(name="qkp", bufs=3) as qkp,
            tc.tile_pool(name="work", bufs=4) as work,
            tc.tile_pool(name="vsm", bufs=8) as vsm,
            tc.tile_pool(name="outp", bufs=2) as outp,
            tc.tile_pool(name="ps_qk", bufs=2, space="PSUM") as ps_qk,
            tc.tile_pool(name="ps_v", bufs=2, space="PSUM") as ps_v,
            tc.tile_pool(name="ps_sc", bufs=2, space="PSUM") as ps_sc,
            tc.tile_pool(name="ps_oa", bufs=2, space="PSUM") as ps_oa,
        ):
            # --- setup DMAs. All HWDGE DMAs issued via nc.sync share ONE
            # FIFO ring (qSPDynamicHW): a kt-shift queued behind N pending
            # x loads waits for ALL of them. So loads are issued just-in-
            # time (2-group lookahead) rather than all up-front, keeping
            # <=1 load ahead of each kt shift in the FIFO. ---
            xg_tiles = {}

            def load_group(g, split=False):
                xg = xin.tile([128, CCH, NB, T], bf16, tag="xg", name=f"xg{g}")
                if split:  # 2-batch halves so pair-0 compute starts sooner
                    for hb in (0, 1):
                        b0 = g * NB + 2 * hb
                        nc.sync.dma_start(
                            out=xg[:, :, 2 * hb:2 * hb + 2, :],
                            in_=xt_d[:, :, b0:b0 + 2, :].rearrange(
                                "k c b t -> c k b t"))
                else:
                    nc.sync.dma_start(
                        out=xg,
                        in_=xt_d[:, :, g * NB:(g + 1) * NB, :].rearrange(
                            "k c b t -> c k b t"))
                xg_tiles[g] = xg

            wqk = singles.tile([128, CCH, 128], bf16)
            nc.sync.dma_start(out=wqk, in_=wqk_d.rearrange("k c h -> c k h"))
            load_group(0, split=True)
            wv = singles.tile([128, CCH, H], bf16)
            nc.sync.dma_start(out=wv, in_=wv_d.rearrange("k c h -> c k h"))
            tri2 = singles.tile([128, 256], bf16)
            nc.sync.dma_start(out=tri2, in_=tri_d.rearrange("p t -> p t"))
            load_group(1)
            load_group(2)

            # --- PE warmup: HAM un-throttles (1.2->2.4GHz) only after ~3.4us
            # of sustained matmul activity. Burn garbage matmuls (uninit
            # SBUF -> dead PSUM slot, reset by the first real start=True)
            # while the first x load is in flight, so real MMs run warm. ---
            scratch = singles.tile([128, 2, 256], bf16)
            nc.gpsimd.memset(scratch, 0.0)
            warm_ps = ps_qk.tile([128, 2, T], fp32, tag="qk", name="warmps")
            for _ in range(WARMUP_MM):
                nc.tensor.matmul(
                    warm_ps[:, 0, :], lhsT=scratch[:, 0, 0:128],
                    rhs=scratch[:, 0, :],
                    start=True, stop=True, skip_group_check=True)

            qk_tiles = {}  # group -> (qk_sb, kt_sb)
            va_tiles = {}  # pair -> va_pair
            mexp_tiles = {}  # pair -> (mexp_sl0, mexp_sl1)
            oa_tiles = {}  # pair -> oa_pair
            osb_tiles = {}  # super-group -> osb

            def proj(p):
                """qkT + v projections for pair p (batches 2p, 2p+1)."""
                g, u = divmod(p, 2)
                xg = xg_tiles[g]
                if u == 0:
                    if g + 3 < NG:
                        load_group(g + 3)
                    qk_sb = qkp.tile([128, NB, T], bf16, tag="qk_sb",
                                     name=f"qksb{g}")
                    qk_tiles[g] = [qk_sb, None]
                qk_sb = qk_tiles[g][0]
                qk_ps = ps_qk.tile([128, 2, T], fp32, tag="qk", name=f"qkps{p}")
                for cc in range(CCH):
                    nc.tensor.matmul(
                        qk_ps,
                        lhsT=wqk[:, cc, :],
                        rhs=xg[:, cc, 2 * u:2 * u + 2, :],
                        start=(cc == 0), stop=(cc == CCH - 1),
                    )
                # split the PSUM->SBUF copy by batch across DVE and ACT
                # (per-partition-serial engines: cost scales with cols only)
                nc.vector.tensor_copy(qk_sb[:, 2 * u, :], qk_ps[:, 0, :])
                nc.scalar.copy(qk_sb[:, 2 * u + 1, :], qk_ps[:, 1, :])
                v_ps = ps_v.tile([128, 2, TCH, H], fp32, tag="v", name=f"vps{p}")
                for sl in range(2):
                    bi = 2 * u + sl
                    for tc2 in range(TCH):
                        for cc in range(CCH):
                            nc.tensor.matmul(
                                v_ps[:, sl, tc2, :],
                                lhsT=xg[:, cc, bi, tc2 * 128:(tc2 + 1) * 128],
                                rhs=wv[:, cc, :],
                                start=(cc == 0), stop=(cc == CCH - 1),
                            )
                va = vsm.tile([128, 2, TCH, H + 1], bf16, tag="va",
                              name=f"va{p}")
                nc.gpsimd.memset(va[:, :, :, H:H + 1], 1.0)
                nc.vector.tensor_copy(va[:, :, :, 0:H], v_ps)
                va_tiles[p] = va
                if u == 1:
                    # kT of both pairs -> partition base 0 (one DMA per group)
                    kt_sb = qkp.tile([64, NB, T], bf16, tag="kt_sb",
                                     name=f"ktsb{g}")
                    nc.sync.dma_start(out=kt_sb, in_=qk_sb[64:128, :, :])
                    qk_tiles[g][1] = kt_sb

            def scores(p):
                """scoresT + exp + causal mask for pair p."""
                g, u = divmod(p, 2)
                qk_sb, kt_sb = qk_tiles[g]
                mexps = []
                for sl in range(2):
                    bg = 2 * u + sl
                    # cols 0:128 = (s0,t0) diag, 128:256 = (s1,t1) diag,
                    # 256:384 = (s0,t1) off-diag -> mask is ONE mul on 0:256
                    sc_ps = ps_sc.tile([128, 3 * 128], fp32, tag="sc",
                                       name=f"scps{p}_{sl}")
                    nc.tensor.matmul(
                        sc_ps[:, 0:128],
                        lhsT=kt_sb[:, bg, 0:128],
                        rhs=qk_sb[0:64, bg, 0:128],
                        start=True, stop=True,
                    )
                    nc.tensor.matmul(
                        sc_ps[:, 128:256],
                        lhsT=kt_sb[:, bg, 128:256],
                        rhs=qk_sb[0:64, bg, 128:256],
                        start=True, stop=True,
                    )
                    nc.tensor.matmul(
                        sc_ps[:, 256:384],
                        lhsT=kt_sb[:, bg, 0:128],
                        rhs=qk_sb[0:64, bg, 128:256],
                        start=True, stop=True,
                    )
                    mexp = work.tile([128, 3 * 128], bf16, tag="mexp",
                                     name=f"mexp{p}_{sl}")
                    nc.scalar.activation(
                        out=mexp, in_=sc_ps, func=Exp, scale=float(H) ** -0.5)
                    # causal mask: both diag blocks in ONE 1-input GpSimd op
                    # (keep where local col j >= partition s, per 128-block)
                    nc.gpsimd.affine_select(
                        out=mexp[:, 0:256].rearrange("p (a j) -> p a j", a=2),
                        in_=mexp[:, 0:256].rearrange("p (a j) -> p a j", a=2),
                        compare_op=mybir.AluOpType.is_ge,
                        fill=0.0, base=0,
                        pattern=[[0, 2], [1, 128]],
                        channel_multiplier=-1,
                    )
                    mexps.append(mexp)
                mexp_tiles[p] = mexps

            def attv(p):
                """att@v + normalize + osb write for pair p."""
                sg = p // (SG // 2)
                if p % (SG // 2) == 0:
                    osb_tiles[sg] = outp.tile([128, TCH, SG, H], bf16,
                                              tag="osb", name=f"osb{sg}")
                osb = osb_tiles[sg]
                va = va_tiles.pop(p)
                mexps = mexp_tiles.pop(p)
                oa_ps = ps_oa.tile([128, 2, TCH, H + 1], fp32, tag="oa",
                                   name=f"oaps{p}")
                for sl in range(2):
                    mexp = mexps[sl]
                    nc.tensor.matmul(
                        oa_ps[:, sl, 0, :], lhsT=mexp[:, 0:128],
                        rhs=va[:, sl, 0, :], start=True, stop=True,
                    )
                    nc.tensor.matmul(
                        oa_ps[:, sl, 1, :], lhsT=mexp[:, 256:384],
                        rhs=va[:, sl, 0, :], start=True, stop=False,
                    )
                    nc.tensor.matmul(
                        oa_ps[:, sl, 1, :], lhsT=mexp[:, 128:256],
                        rhs=va[:, sl, 1, :], start=False, stop=True,
                    )
                rec = vsm.tile([128, 2, TCH, 1], fp32, tag="rec", name=f"rec{p}")
                nc.vector.reciprocal(rec, oa_ps[:, :, :, H:H + 1])
                # one broadcast-multiply normalizes the whole pair: out AP is
                # (tc, slot=sl, h), inputs rearranged (sl, tc -> tc, sl)
                s0 = (2 * p) % SG
                nc.vector.tensor_mul(
                    osb[:, :, s0:s0 + 2, :],
                    oa_ps[:, :, :, 0:H].rearrange("p a b h -> p b a h"),
                    rec.rearrange("p a b o -> p b a o").broadcast_to(
                        [128, TCH, 2, H]),
                )
                last_sg = NP // (SG // 2) - 1
                if sg == last_sg and s0 + 1 == SG // 2 - 1:
                    nc.sync.dma_start(
                        out=out_d[:, sg * SG:sg * SG + SG // 2, :].rearrange(
                            "(c p) b h -> p c b h", p=128),
                        in_=osb[:, :, 0:SG // 2, :],
                    )
                elif sg == last_sg and s0 + 1 == SG - 1:
                    nc.sync.dma_start(
                        out=out_d[:, sg * SG + SG // 2:(sg + 1) * SG, :]
                        .rearrange("(c p) b h -> p c b h", p=128),
                        in_=osb[:, :, SG // 2:SG, :],
                    )
                elif s0 + 1 == SG - 1:
                    nc.sync.dma_start(
                        out=out_d[:, sg * SG:(sg + 1) * SG, :].rearrange(
                            "(c p) b h -> p c b h", p=128),
                        in_=osb,
                    )

            # attention-first inside each iteration: scores/attv consume
            # tiles produced >=1 full iteration ago (PE never waits on
            # ACT/DVE/DMA), and they gap-fill when proj is x-load-bound.
            for p in range(NP + LAG_AV):
                if LAG_SC <= p < NP + LAG_SC:
                    scores(p - LAG_SC)
                if p >= LAG_AV:
                    attv(p - LAG_AV)
                if p < NP:
                    proj(p)
    nc.compile()
    return nc


def _get_nc():
    if "nc" not in _CACHE:
        _CACHE["nc"] = _build()
    return _CACHE["nc"]


def _prep_inputs(x, Wq, Wk, Wv):
    """Host-side marshaling: shard + cast + transpose to device layouts."""
    x = np.asarray(x, dtype=np.float32)
    wqk = np.ascontiguousarray(
        np.concatenate([np.asarray(Wq, np.float32), np.asarray(Wk, np.float32)],
                       axis=1).astype(BF16).reshape(CCH, 128, 128))
    wv = np.ascontiguousarray(
        np.asarray(Wv, np.float32).astype(BF16).reshape(CCH, 128, H))
    tri = np.triu(np.ones((128, 128), dtype=BF16))
    tri2 = np.ascontiguousarray(np.concatenate([tri, tri], axis=1))
    in_maps = []
    for i in range(NCORES):
        xs = x[i * BPC:(i + 1) * BPC]  # [32, 256, 384]
        xt = np.ascontiguousarray(
            xs.transpose(2, 0, 1).astype(BF16).reshape(CCH, 128, BPC, T))
        in_maps.append({"xt": xt, "wqk": wqk, "wv": wv, "tri2": tri2})
    return in_maps


def kernel(x, Wq, Wk, Wv):
    from concourse.bass_utils import run_bass_kernel_spmd

    nc = _get_nc()
    in_maps = _prep_inputs(x, Wq, Wk, Wv)
    res = run_bass_kernel_spmd(nc, in_maps, list(range(NCORES)))
    # out per core: [T, BPC, H] bf16 -> [BPC, T, H] f32
    return np.concatenate(
        [np.asarray(res.results[i]["out"]).astype(np.float32).transpose(1, 0, 2)
         for i in range(NCORES)], axis=0)


# revision 20
# speedup vs baseline: 1.0123x; 1.0123x over previous
"""Single-head causal attention kernel for TRN2 (8 NeuronCores, data-parallel).

Problem: x[256,256,384] f32, Wq/Wk/Wv[384,64] f32 ->
  out = softmax(mask((x@Wq)(x@Wk)^T/8)) @ (x@Wv)  [256,256,64] f32

Sharding: batch 256 -> 8 cores x 32 batches. Weights replicated.

Host-side marshaling (inside kernel(), per core):
  x slice  -> bf16, transposed to xT layout [cc, c, b, t]  (c = cc*128+c')
  Wq|Wk    -> packed bf16 [cc, c, 128] (cols 0:64 q, 64:128 k)
  Wv       -> bf16 [cc, c, 64]
  tri2     -> [128,256] bf16 = [tri|tri] causal keep-mask (diag blocks)
  out      <- bf16 [t, b, h], host transposes back to [b, t, h] fp32

The xT layout kills all on-device PE transposes (projections contract C, so
both operands want C on partitions) and halves x HBM traffic vs fp32. All
DMA descriptors are >=1KB contiguous runs; all 8 x-group loads are issued
up-front (no waits) so the SP sequencer never head-of-line blocks them.

Per-core dataflow (bf16 matmuls, fp32 PSUM), per batch-pair:
  qkT [h2=128, 2, 256] = wqk^T @ xT    3 MMs (512-col rhs), PSUM->SBUF (DVE)
  kT -> partition base 0 via SBUF->SBUF DMA (PE operands must share array
        rows; engines cannot shift partitions, DMA can) - one per group
  v   [t, 64] = x @ Wv                 12 MMs (64-col rhs), ACT copy to va
  scT [s, t]  diag-adjacent cols:      6 MMs: 0:128=(s0,t0) 128:256=(s1,t1)
        256:384=(s0,t1); causal mask = ONE GpSimd mul of cols 0:256 by tri2
  mexp = exp(scT/8) on ACT (bf16 out)
  oa  [t, 65] = mexp^T @ [v|1]         6 MMs; rowsum lands in col 64
  normalize: DVE pair-reciprocal + broadcast-mul -> osb bf16

3-stage software pipeline keeps PE dense (HAM @ 2.4GHz): at iteration p the
kernel emits proj(p), scores(p-3), attv(p-4); cross-engine results are
always >=1 full iteration old when PE consumes them.
"""

import numpy as np
import ml_dtypes

B, T, C, H = 256, 256, 384, 64
NCORES = 8
BPC = B // NCORES  # 32 batches per core
CCH = C // 128  # 3 contraction chunks
TCH = T // 128  # 2 t-chunks
NB = 4  # batches per x-load group
NG = BPC // NB  # 8 groups
NP = BPC // 2  # 16 pairs
SG = 8  # batches per out-store super-group
LAG_SC = 3  # scores stage lag (pairs)
LAG_AV = 5  # attv stage lag (pairs)
WARMUP_MM = 17  # garbage 256-col matmuls to push HAM to 2.4GHz while x loads

BF16 = ml_dtypes.bfloat16

_CACHE = {}


def _build():
    import concourse.mybir as mybir
    import concourse.tile as tile
    from concourse import bacc

    fp32 = mybir.dt.float32
    bf16 = mybir.dt.bfloat16
    Exp = mybir.ActivationFunctionType.Exp
    Copy = mybir.ActivationFunctionType.Copy

    nc = bacc.Bacc()
    xt_d = nc.declare_dram_parameter("xt", [CCH, 128, BPC, T], bf16, isOutput=False)
    wqk_d = nc.declare_dram_parameter("wqk", [CCH, 128, 128], bf16, isOutput=False)
    wv_d = nc.declare_dram_parameter("wv", [CCH, 128, H], bf16, isOutput=False)
    tri_d = nc.declare_dram_parameter("tri2", [128, 256], bf16, isOutput=False)
    out_d = nc.declare_dram_parameter("out", [T, BPC, H], bf16, isOutput=True)

    with tile.TileContext(nc) as tc:
        with (
            tc.tile_pool(name="singles", bufs=1) as singles,
            tc.tile_pool(name="xin", bufs=8) as xin,
            tc.tile_pool(name="qkp", bufs=3) as qkp,
            tc.tile_pool(name="work", bufs=4) as work,
            tc.tile_pool(name="vsm", bufs=8) as vsm,
            tc.tile_pool(name="outp", bufs=2) as outp,
            tc.tile_pool(name="ps_qk", bufs=2, space="PSUM") as ps_qk,
            tc.tile_pool(name="ps_v", bufs=2, space="PSUM") as ps_v,
            tc.tile_pool(name="ps_sc", bufs=2, space="PSUM") as ps_sc,
            tc.tile_pool(name="ps_oa", bufs=2, space="PSUM") as ps_oa,
        ):
            # --- setup DMAs. All HWDGE DMAs issued via nc.sync share ONE
            # FIFO ring (qSPDynamicHW): a kt-shift queued behind N pending
            # x loads waits for ALL of them. So loads are issued just-in-
            # time (2-group lookahead) rather than all up-front, keeping
            # <=1 load ahead of each kt shift in the FIFO. ---
            xg_tiles = {}

            def load_group(g, split=False):
                xg = xin.tile([128, CCH, NB, T], bf16, tag="xg", name=f"xg{g}")
                if split:  # 2-batch halves so pair-0 compute starts sooner
                    for hb in (0, 1):
                        b0 = g * NB + 2 * hb
                        nc.sync.dma_start(
                            out=xg[:, :, 2 * hb:2 * hb + 2, :],
                            in_=xt_d[:, :, b0:b0 + 2, :].rearrange(
                                "k c b t -> c k b t"))
                else:
                    nc.sync.dma_start(
                        out=xg,
                        in_=xt_d[:, :, g * NB:(g + 1) * NB, :].rearrange(
                            "k c b t -> c k b t"))
                xg_tiles[g] = xg

            wqk = singles.tile([128, CCH, 128], bf16)
            nc.sync.dma_start(out=wqk, in_=wqk_d.rearrange("k c h -> c k h"))
            load_group(0, split=True)
            wv = singles.tile([128, CCH, H], bf16)
            nc.sync.dma_start(out=wv, in_=wv_d.rearrange("k c h -> c k h"))
            tri2 = singles.tile([128, 256], bf16)
            nc.sync.dma_start(out=tri2, in_=tri_d.rearrange("p t -> p t"))
            load_group(1)
            load_group(2)

            # --- PE warmup: HAM un-throttles (1.2->2.4GHz) only after ~3.4us
            # of sustained matmul activity. Burn garbage matmuls (uninit
            # SBUF -> dead PSUM slot, reset by the first real start=True)
            # while the first x load is in flight, so real MMs run warm. ---
            scratch = singles.tile([128, 2, 256], bf16)
            nc.gpsimd.memset(scratch, 0.0)
            warm_a = ps_qk.tile([128, 2, T], fp32, tag="qk", name="warmpsa")
            warm_b = ps_qk.tile([128, 2, T], fp32, tag="qk", name="warmpsb")
            for i in range(WARMUP_MM):
                nc.tensor.matmul(
                    (warm_a if i % 2 == 0 else warm_b)[:, 0, :],
                    lhsT=scratch[:, 0, 0:128], rhs=scratch[:, 0, :],
                    start=True, stop=True, skip_group_check=True)

            qk_tiles = {}  # group -> (qk_sb, kt_sb)
            va_tiles = {}  # pair -> va_pair
            mexp_tiles = {}  # pair -> (mexp_sl0, mexp_sl1)
            oa_tiles = {}  # pair -> oa_pair
            osb_tiles = {}  # super-group -> osb

            def proj(p):
                """qkT + v projections for pair p (batches 2p, 2p+1)."""
                g, u = divmod(p, 2)
                xg = xg_tiles[g]
                if u == 0:
                    if g + 3 < NG:
                        load_group(g + 3)
                    qk_sb = qkp.tile([128, NB, T], bf16, tag="qk_sb",
                                     name=f"qksb{g}")
                    qk_tiles[g] = [qk_sb, None]
                qk_sb = qk_tiles[g][0]
                qk_ps = ps_qk.tile([128, 2, T], fp32, tag="qk", name=f"qkps{p}")
                for cc in range(CCH):
                    nc.tensor.matmul(
                        qk_ps,
                        lhsT=wqk[:, cc, :],
                        rhs=xg[:, cc, 2 * u:2 * u + 2, :],
                        start=(cc == 0), stop=(cc == CCH - 1),
                    )
                # split the PSUM->SBUF copy by batch across DVE and ACT
                # (per-partition-serial engines: cost scales with cols only)
                nc.vector.tensor_copy(qk_sb[:, 2 * u, :], qk_ps[:, 0, :])
                nc.scalar.copy(qk_sb[:, 2 * u + 1, :], qk_ps[:, 1, :])
                v_ps = ps_v.tile([128, 2, TCH, H], fp32, tag="v", name=f"vps{p}")
                for sl in range(2):
                    bi = 2 * u + sl
                    for tc2 in range(TCH):
                        for cc in range(CCH):
                            nc.tensor.matmul(
                                v_ps[:, sl, tc2, :],
                                lhsT=xg[:, cc, bi, tc2 * 128:(tc2 + 1) * 128],
                                rhs=wv[:, cc, :],
                                start=(cc == 0), stop=(cc == CCH - 1),
                            )
                va = vsm.tile([128, 2, TCH, H + 1], bf16, tag="va",
                              name=f"va{p}")
                nc.gpsimd.memset(va[:, :, :, H:H + 1], 1.0)
                nc.vector.tensor_copy(va[:, :, :, 0:H], v_ps)
                va_tiles[p] = va
                if u == 1:
                    # kT of both pairs -> partition base 0 (one DMA per group)
                    kt_sb = qkp.tile([64, NB, T], bf16, tag="kt_sb",
                                     name=f"ktsb{g}")
                    nc.sync.dma_start(out=kt_sb, in_=qk_sb[64:128, :, :])
                    qk_tiles[g][1] = kt_sb

            def scores(p):
                """scoresT + exp + causal mask for pair p."""
                g, u = divmod(p, 2)
                qk_sb, kt_sb = qk_tiles[g]
                mexps = []
                for sl in range(2):
                    bg = 2 * u + sl
                    # cols 0:128 = (s0,t0) diag, 128:256 = (s1,t1) diag,
                    # 256:384 = (s0,t1) off-diag -> mask is ONE mul on 0:256
                    sc_ps = ps_sc.tile([128, 3 * 128], fp32, tag="sc",
                                       name=f"scps{p}_{sl}")
                    nc.tensor.matmul(
                        sc_ps[:, 0:128],
                        lhsT=kt_sb[:, bg, 0:128],
                        rhs=qk_sb[0:64, bg, 0:128],
                        start=True, stop=True,
                    )
                    nc.tensor.matmul(
                        sc_ps[:, 128:256],
                        lhsT=kt_sb[:, bg, 128:256],
                        rhs=qk_sb[0:64, bg, 128:256],
                        start=True, stop=True,
                    )
                    nc.tensor.matmul(
                        sc_ps[:, 256:384],
                        lhsT=kt_sb[:, bg, 0:128],
                        rhs=qk_sb[0:64, bg, 128:256],
                        start=True, stop=True,
                    )
                    mexp = work.tile([128, 3 * 128], bf16, tag="mexp",
                                     name=f"mexp{p}_{sl}")
                    nc.scalar.activation(
                        out=mexp, in_=sc_ps, func=Exp, scale=float(H) ** -0.5)
                    # causal mask: both diag blocks in ONE 1-input GpSimd op
                    # (keep where local col j >= partition s, per 128-block)
                    nc.gpsimd.affine_select(
                        out=mexp[:, 0:256].rearrange("p (a j) -> p a j", a=2),
                        in_=mexp[:, 0:256].rearrange("p (a j) -> p a j", a=2),
                        compare_op=mybir.AluOpType.is_ge,
                        fill=0.0, base=0,
                        pattern=[[0, 2], [1, 128]],
                        channel_multiplier=-1,
                    )
                    mexps.append(mexp)
                mexp_tiles[p] = mexps

            def attv(p):
                """att@v + normalize + osb write for pair p."""
                sg = p // (SG // 2)
                if p % (SG // 2) == 0:
                    osb_tiles[sg] = outp.tile([128, TCH, SG, H], bf16,
                                              tag="osb", name=f"osb{sg}")
                osb = osb_tiles[sg]
                va = va_tiles.pop(p)
                mexps = mexp_tiles.pop(p)
                oa_ps = ps_oa.tile([128, 2, TCH, H + 1], fp32, tag="oa",
                                   name=f"oaps{p}")
                for sl in range(2):
                    mexp = mexps[sl]
                    nc.tensor.matmul(
                        oa_ps[:, sl, 0, :], lhsT=mexp[:, 0:128],
                        rhs=va[:, sl, 0, :], start=True, stop=True,
                    )
                    nc.tensor.matmul(
                        oa_ps[:, sl, 1, :], lhsT=mexp[:, 256:384],
                        rhs=va[:, sl, 0, :], start=True, stop=False,
                    )
                    nc.tensor.matmul(
                        oa_ps[:, sl, 1, :], lhsT=mexp[:, 128:256],
                        rhs=va[:, sl, 1, :], start=False, stop=True,
                    )
                rec = vsm.tile([128, 2, TCH, 1], fp32, tag="rec", name=f"rec{p}")
                nc.vector.reciprocal(rec, oa_ps[:, :, :, H:H + 1])
                # one broadcast-multiply normalizes the whole pair: out AP is
                # (tc, slot=sl, h), inputs rearranged (sl, tc -> tc, sl)
                s0 = (2 * p) % SG
                nc.vector.tensor_mul(
                    osb[:, :, s0:s0 + 2, :],
                    oa_ps[:, :, :, 0:H].rearrange("p a b h -> p b a h"),
                    rec.rearrange("p a b o -> p b a o").broadcast_to(
                        [128, TCH, 2, H]),
                )
                last_sg = NP // (SG // 2) - 1
                if sg == last_sg and s0 + 1 == SG // 2 - 1:
                    nc.sync.dma_start(
                        out=out_d[:, sg * SG:sg * SG + SG // 2, :].rearrange(
                            "(c p) b h -> p c b h", p=128),
                        in_=osb[:, :, 0:SG // 2, :],
                    )
                elif sg == last_sg and s0 + 1 == SG - 1:
                    nc.sync.dma_start(
                        out=out_d[:, sg * SG + SG // 2:(sg + 1) * SG, :]
                        .rearrange("(c p) b h -> p c b h", p=128),
                        in_=osb[:, :, SG // 2:SG, :],
                    )
                elif s0 + 1 == SG - 1:
                    nc.sync.dma_start(
                        out=out_d[:, sg * SG:(sg + 1) * SG, :].rearrange(
                            "(c p) b h -> p c b h", p=128),
                        in_=osb,
                    )

            # attention-first inside each iteration: scores/attv consume
            # tiles produced >=1 full iteration ago (PE never waits on
            # ACT/DVE/DMA), and they gap-fill when proj is x-load-bound.
            for p in range(NP + LAG_AV):
                if LAG_SC <= p < NP + LAG_SC:
                    scores(p - LAG_SC)
                if p >= LAG_AV:
                    attv(p - LAG_AV)
                if p < NP:
                    proj(p)
    nc.compile()
    return nc


def _get_nc():
    if "nc" not in _CACHE:
        _CACHE["nc"] = _build()
    return _CACHE["nc"]


def _prep_inputs(x, Wq, Wk, Wv):
    """Host-side marshaling: shard + cast + transpose to device layouts."""
    x = np.asarray(x, dtype=np.float32)
    wqk = np.ascontiguousarray(
        np.concatenate([np.asarray(Wq, np.float32), np.asarray(Wk, np.float32)],
                       axis=1).astype(BF16).reshape(CCH, 128, 128))
    wv = np.ascontiguousarray(
        np.asarray(Wv, np.float32).astype(BF16).reshape(CCH, 128, H))
    tri = np.triu(np.ones((128, 128), dtype=BF16))
    tri2 = np.ascontiguousarray(np.concatenate([tri, tri], axis=1))
    in_maps = []
    for i in range(NCORES):
        xs = x[i * BPC:(i + 1) * BPC]  # [32, 256, 384]
        xt = np.ascontiguousarray(
            xs.transpose(2, 0, 1).astype(BF16).reshape(CCH, 128, BPC, T))
        in_maps.append({"xt": xt, "wqk": wqk, "wv": wv, "tri2": tri2})
    return in_maps


def kernel(x, Wq, Wk, Wv):
    from concourse.bass_utils import run_bass_kernel_spmd

    nc = _get_nc()
    in_maps = _prep_inputs(x, Wq, Wk, Wv)
    res = run_bass_kernel_spmd(nc, in_maps, list(range(NCORES)))
    # out per core: [T, BPC, H] bf16 -> [BPC, T, H] f32
    return np.concatenate(
        [np.asarray(res.results[i]["out"]).astype(np.float32).transpose(1, 0, 2)
         for i in range(NCORES)], axis=0)
